# revision 35
# baseline (speedup 1.0000x reference)
"""Multi-head attention (B=4, S=2048, D=1024, H=16, causal) on 8 trn2 NeuronCores.

Sharding: tensor-parallel over heads. Core c owns heads {2c, 2c+1} = model dims
[c*128, (c+1)*128).

Per-core pipeline (all matmul inputs bf16, fp32 PSUM accumulation):
  A) Q/K/V projections in transposed layout  X_c [128 dims, rows]
     (lhsT = W.T chunk stationary, rhs = host-pretransposed input stream).
  B) V transposed back to row-major via PE transpose and augmented with a
     64-wide ones block -> V_aug tiles [128 k-rows, 192].
  C) Attention per (batch, 512-q window, 128-k block), causal blocks only:
     scoresT [k,q] via 2-head row-packed matmuls, exp (scale=1/8 folded in,
     no max subtraction - scores are O(1) by construction), triangular mask
     on diagonal blocks, PV accumulation with the ones-augmented V so PSUM
     rows carry both context and the softmax denominator replicated 64x.
     Divide on DVE -> ctxT [dims, q] bf16.
  D) Output projection -> outT [1024 o, q] bf16 partials per core; host
     transposes, upcasts and reduces the 8 partials.

Scheduling: single-pass emission with a filler FIFO. Attention windows run
with one-block lookahead (sc(k+1) issued before pv(k) so the exp on ACT is
off the PE critical path); between attention matmuls the scheduler pops
"filler" PE micro-ops (next batch's projections, previous windows' output
projections) so the PE never idles waiting on ACT/DVE.
"""

import os
import sys
from collections import deque
from contextlib import ExitStack

sys.path.insert(0, "/opt/trn_rl_repo")

import numpy as np
import ml_dtypes

import concourse.bass as bass
import concourse.bacc as bacc
import concourse.mybir as mybir
import concourse.tile as tile
from concourse.bass_utils import run_bass_kernel_spmd

BF = mybir.dt.bfloat16
F32 = mybir.dt.float32
bf16 = ml_dtypes.bfloat16

B, S, D = 4, 2048, 1024
R = B * S  # 8192
NCORES = 8
QW = 512  # q-window
NKB = S // 128  # 16 k-blocks per batch

_CACHE: dict = {}


def _build_program() -> bass.Bass:
    nc = bacc.Bacc(None, num_devices=NCORES)
    xqT = nc.dram_tensor("xqT", [D, R], BF, kind="ExternalInput")
    xkT = nc.dram_tensor("xkT", [D, R], BF, kind="ExternalInput")
    xvT = nc.dram_tensor("xvT", [D, R], BF, kind="ExternalInput")
    wq = nc.dram_tensor("wq", [D, 128], BF, kind="ExternalInput")
    wk = nc.dram_tensor("wk", [D, 128], BF, kind="ExternalInput")
    wv = nc.dram_tensor("wv", [D, 128], BF, kind="ExternalInput")
    wo = nc.dram_tensor("wo", [128, D], BF, kind="ExternalInput")
    bq = nc.dram_tensor("bq", [128, 1], F32, kind="ExternalInput")
    bk = nc.dram_tensor("bk", [128, 1], F32, kind="ExternalInput")
    bv = nc.dram_tensor("bv", [128, 1], F32, kind="ExternalInput")
    tri = nc.dram_tensor("tri", [128, 2, 128], BF, kind="ExternalInput")
    ident = nc.dram_tensor("ident", [128, 128], BF, kind="ExternalInput")
    out_ext = nc.dram_tensor("out", [D, R], BF, kind="ExternalOutput")

    EXP = mybir.ActivationFunctionType.Exp
    LN = mybir.ActivationFunctionType.Ln

    with ExitStack() as ctx:
        tc = ctx.enter_context(tile.TileContext(nc))
        singles = ctx.enter_context(tc.tile_pool(name="singles", bufs=1))
        stage = ctx.enter_context(tc.tile_pool(name="stage", bufs=20))
        qkv = ctx.enter_context(tc.tile_pool(name="qkv", bufs=2))
        vst = ctx.enter_context(tc.tile_pool(name="vst", bufs=3))
        exps = ctx.enter_context(tc.tile_pool(name="exps", bufs=8))
        divp = ctx.enter_context(tc.tile_pool(name="divp", bufs=6))
        outp = ctx.enter_context(tc.tile_pool(name="outp", bufs=2))
        ps_proj = ctx.enter_context(tc.tile_pool(name="ps_proj", bufs=2, space="PSUM"))
        ps_sc = ctx.enter_context(tc.tile_pool(name="ps_sc", bufs=2, space="PSUM"))
        ps_pv = ctx.enter_context(tc.tile_pool(name="ps_pv", bufs=2, space="PSUM"))

        # resident constants
        wq_sb = singles.tile([128, 8, 128], BF, name="wq_sb")
        wk_sb = singles.tile([128, 8, 128], BF, name="wk_sb")
        wv_sb = singles.tile([128, 8, 128], BF, name="wv_sb")
        nc.sync.dma_start(wq_sb[:], wq[:, :].rearrange("(f p) m -> p f m", p=128))
        nc.sync.dma_start(wk_sb[:], wk[:, :].rearrange("(f p) m -> p f m", p=128))
        nc.sync.dma_start(wv_sb[:], wv[:, :].rearrange("(f p) m -> p f m", p=128))
        wo_sb = singles.tile([128, D], BF, name="wo_sb")
        nc.sync.dma_start(wo_sb[:], wo[:, :])
        bq_sb = singles.tile([128, 1], F32, name="bq_sb")
        bk_sb = singles.tile([128, 1], F32, name="bk_sb")
        bv_sb = singles.tile([128, 1], F32, name="bv_sb")
        nc.sync.dma_start(bq_sb[:], bq[:, :])
        nc.sync.dma_start(bk_sb[:], bk[:, :])
        nc.sync.dma_start(bv_sb[:], bv[:, :])
        tri_sb = singles.tile([128, 2, 128], BF, name="tri_sb")
        nc.sync.dma_start(tri_sb[:], tri[:, :, :])
        id_sb = singles.tile([128, 128], BF, name="id_sb")
        nc.sync.dma_start(id_sb[:], ident[:, :])

        warm_sb = singles.tile([128, 512], BF, name="warm_sb")
        nc.vector.memset(warm_sb[:], 0.0)
        warm_ps = ps_proj.tile([128, 512], F32, tag="proj", name="warm_ps")
        for wi in range(24):
            nc.tensor.matmul(
                warm_ps[:],
                warm_sb[:, 0:128],
                warm_sb[:],
                start=(wi == 0),
                stop=(wi == 23),
            )

        tiles = {}
        bslabs = {}
        fillers: deque = deque()

        def pump(n):
            while n > 0 and fillers:
                try:
                    next(fillers[0])
                    n -= 1
                except StopIteration:
                    fillers.popleft()

        def flush_all():
            while fillers:
                try:
                    next(fillers[0])
                except StopIteration:
                    fillers.popleft()

        def alloc_batch(b):
            q_sb = qkv.tile([128, S], BF, tag="q_sb", name=f"q_sb{b}")
            k_sb = qkv.tile([128, S], BF, tag="k_sb", name=f"k_sb{b}")
            v_aug = qkv.tile([128, NKB, 192], BF, tag="v_aug", name=f"v_aug{b}")
            nc.vector.memset(v_aug[:, :, 64:128], 1.0)
            tiles[b] = (q_sb, k_sb, v_aug)

        def projection_microops(b, which, half):
            # which: 0=q, 1=k, 2=v; half: 0/1 = 1024-row sector. Generator:
            # one PE matmul per next(). f-outer over a pair of 512-col psum
            # groups so consecutive matmuls share lhsT.
            if b not in tiles:
                alloc_batch(b)
            q_sb, k_sb, v_aug = tiles[b]
            xT, w_sb, b_sb = (
                (xqT, wq_sb, bq_sb),
                (xkT, wk_sb, bk_sb),
                (xvT, wv_sb, bv_sb),
            )[which]
            st = {}
            for f in range(8):
                s_t = stage.tile([128, S // 2], BF, tag="stage")
                nc.sync.dma_start(
                    s_t[:],
                    xT[
                        f * 128 : (f + 1) * 128,
                        b * S + half * (S // 2) : b * S + (half + 1) * (S // 2),
                    ],
                )
                st[f] = s_t
            pa = ps_proj.tile(
                [128, 512], F32, tag="proj", name=f"pa{b}_{half}_{which}"
            )
            pb = ps_proj.tile(
                [128, 512], F32, tag="proj", name=f"pb{b}_{half}_{which}"
            )
            for f in range(8):
                nc.tensor.matmul(
                    pa[:],
                    w_sb[:, f, :],
                    st[f][:, 0:512],
                    start=(f == 0),
                    stop=(f == 7),
                )
                yield
                nc.tensor.matmul(
                    pb[:],
                    w_sb[:, f, :],
                    st[f][:, 512:1024],
                    start=(f == 0),
                    stop=(f == 7),
                )
                yield
            t0 = 2 * half
            if which == 0:
                nc.vector.tensor_scalar_add(
                    q_sb[:, t0 * 512 : (t0 + 1) * 512], pa[:], bq_sb[:]
                )
                nc.vector.tensor_scalar_add(
                    q_sb[:, (t0 + 1) * 512 : (t0 + 2) * 512], pb[:], bq_sb[:]
                )
            elif which == 1:
                nc.vector.tensor_scalar_add(
                    k_sb[:, t0 * 512 : (t0 + 1) * 512], pa[:], bk_sb[:]
                )
                nc.vector.tensor_scalar_add(
                    k_sb[:, (t0 + 1) * 512 : (t0 + 2) * 512], pb[:], bk_sb[:]
                )
            else:
                v_sts = {}
                for t, ps in ((t0, pa), (t0 + 1, pb)):
                    v_st = vst.tile([128, 512], BF, tag="v_st")
                    nc.vector.tensor_scalar_add(v_st[:], ps[:], bv_sb[:])
                    v_sts[t] = v_st
                for t in (t0, t0 + 1):
                    for s4 in range(4):
                        pst = ps_proj.tile(
                            [128, 128], BF, tag="proj", name=f"pst{b}_{t}_{s4}"
                        )
                        nc.tensor.transpose(
                            pst[:], v_sts[t][:, s4 * 128 : (s4 + 1) * 128], id_sb[:]
                        )
                        tt = t * 4 + s4
                        nc.vector.tensor_copy(v_aug[:, tt, 0:64], pst[:, 0:64])
                        nc.vector.tensor_copy(v_aug[:, tt, 128:192], pst[:, 64:128])
                        yield

        def outproj_windows(wins, act_mod=4):
            # Output projection for one or more q-windows; back-to-back
            # matmuls per ob share the wo lhsT chunk. One DMA per window.
            # act_mod: every act_mod-th PSUM evac copy goes to ACT, rest DVE.
            ots = [
                outp.tile([128, 8, 512], BF, tag="ot", name=f"ot{win}")
                for _, win in wins
            ]
            for ob in range(8):
                pos = []
                for (ctx_t, win), ot in zip(wins, ots):
                    po = ps_proj.tile(
                        [128, 512], F32, tag="proj", name=f"po{win}_{ob}"
                    )
                    nc.tensor.matmul(
                        po[:],
                        wo_sb[:, ob * 128 : (ob + 1) * 128],
                        ctx_t[:],
                        start=True,
                        stop=True,
                    )
                    pos.append((po, ot))
                    yield
                for po, ot in pos:
                    if ob % act_mod == act_mod - 1:
                        nc.scalar.copy(ot[:, ob, :], po[:])
                    else:
                        nc.vector.tensor_copy(ot[:, ob, :], po[:])
            for (ctx_t, win), ot in zip(wins, ots):
                nc.gpsimd.dma_start(
                    out_ext[:, win : win + 512].rearrange("(f p) m -> p f m", p=128),
                    ot[:],
                )

        def emit_attention_qb(b, qb):
            q_sb, k_sb, v_aug = tiles[b]
            nk = 4 * qb + 4  # causal: k-blocks 0 .. 4qb+3
            pv0 = ps_pv.tile([128, 512], F32, tag="pv", name=f"pv0_{b}_{qb}")
            pv1 = ps_pv.tile([128, 512], F32, tag="pv", name=f"pv1_{b}_{qb}")
            ets = {}

            def emit_sc(kblk):
                r = kblk - 4 * qb
                q_lo = max(0, r * 128)
                sc = ps_sc.tile([128, 2, 512], F32, tag="sc")
                for h in range(2):
                    nc.tensor.matmul(
                        sc[:, h, q_lo:512],
                        k_sb[h * 64 : (h + 1) * 64, kblk * 128 : (kblk + 1) * 128],
                        q_sb[
                            h * 64 : (h + 1) * 64,
                            qb * 512 + q_lo : (qb + 1) * 512,
                        ],
                        start=True,
                        stop=True,
                        tile_position=(h * 64, 0),
                    )
                et = exps.tile([128, 2, 512], BF, tag="et")
                nc.scalar.activation(
                    et[:, :, q_lo:512], sc[:, :, q_lo:512], EXP, scale=0.125
                )
                if r >= 0:
                    nc.vector.tensor_mul(
                        et[:, :, q_lo : q_lo + 128],
                        et[:, :, q_lo : q_lo + 128],
                        tri_sb[:],
                    )
                ets[kblk] = (et, q_lo)

            def emit_pv(kblk):
                et, q_lo = ets.pop(kblk)
                for h, pv in ((0, pv0), (1, pv1)):
                    nc.tensor.matmul(
                        pv[:, q_lo:512],
                        v_aug[:, kblk, h * 64 : h * 64 + 128],
                        et[:, h, q_lo:512],
                        start=(kblk == 0),
                        stop=(kblk == nk - 1),
                    )

            emit_sc(0)
            pump(3)
            for kblk in range(nk):
                if kblk + 1 < nk:
                    emit_sc(kblk + 1)
                pump(1)
                emit_pv(kblk)
            # evac: pv0 rows[0:64]=ctx~ h0, rows[64:128]=l h0 (replicated)
            #       pv1 rows[0:64]=l h1,   rows[64:128]=ctx~ h1
            # DVE copies free the pv banks ~2.5us after the last
            # accumulation, landing in per-batch slab tiles (cslb/lslb).
            # The 1/l normalize is ONE Ln + ONE Exp + ONE mul over the whole
            # batch slab: the tile scheduler cannot interleave attention
            # exps between them, so the Ln<->Exp act-table swap happens
            # twice per batch instead of twice per window.
            cslb, lslb = bslabs[b]
            nc.vector.tensor_copy(cslb[0:64, qb, :], pv0[0:64, :])
            nc.vector.tensor_copy(lslb[0:64, qb, :], pv0[64:128, :])
            nc.vector.tensor_copy(cslb[64:128, qb, :], pv1[64:128, :])
            nc.vector.tensor_copy(lslb[64:128, qb, :], pv1[0:64, :])

        def emit_norm_cluster(b, q0, n):
            # One Ln + one Exp + one mul over windows [q0, q0+n) of batch b.
            cslb, lslb = bslabs[b]
            lnw = divp.tile([128, 4, 512], F32, tag="lnw", bufs=3)
            nc.scalar.activation(lnw[:, 0:n, :], lslb[:, q0 : q0 + n, :], LN)
            recb = divp.tile([128, 4, 512], BF, tag="recb", bufs=3)
            nc.scalar.activation(recb[:, 0:n, :], lnw[:, 0:n, :], EXP, scale=-1.0)
            ctxb = divp.tile([128, 4, 512], BF, tag="ctxb", bufs=3)
            nc.vector.tensor_mul(
                ctxb[:, 0:n, :], cslb[:, q0 : q0 + n, :], recb[:, 0:n, :]
            )
            return [(ctxb[:, i, :], b * S + (q0 + i) * 512) for i in range(n)]

        # prologue: batch-0 half-0 projections inline (rows 0:1024 of q/k/v);
        # windows 0-1 only need those, so attention starts while half-1
        # streams in as filler work.
        for which in range(3):
            for _ in projection_microops(0, which, 0):
                pass
        b0h1 = [projection_microops(0, which, 1) for which in range(3)]
        for g in b0h1:
            fillers.append(g)
        for b in range(B):
            if b + 1 < B:
                for which in range(3):
                    for half in range(2):
                        fillers.append(projection_microops(b + 1, which, half))
            bslabs[b] = (
                divp.tile([128, 4, 512], BF, tag="cslb", name=f"cslb{b}", bufs=2),
                divp.tile([128, 4, 512], F32, tag="lslb", name=f"lslb{b}", bufs=2),
            )
            for qb in range(S // QW):
                if b == 0 and qb == 2:
                    # windows 2-3 of batch 0 need half-1 q/k/v: drain the
                    # batch-0 half-1 projection fillers first
                    while any(g in fillers for g in b0h1):
                        pump(1)
                emit_attention_qb(b, qb)
                if b == B - 1 and qb in (1, 2):
                    # last batch has no next-batch projections to fill PE
                    # idle: normalize early windows as they finish so their
                    # output projection overlaps the later windows' attention
                    if qb == 1:
                        fillers.append(
                            outproj_windows(emit_norm_cluster(b, 0, 2))
                        )
                    else:
                        fillers.append(
                            outproj_windows(emit_norm_cluster(b, 2, 1))
                        )
            if b == B - 1:
                fillers.append(
                    outproj_windows(emit_norm_cluster(b, 3, 1), act_mod=2)
                )
            else:
                wins = emit_norm_cluster(b, 0, 4)
                fillers.append(outproj_windows(wins[0:2]))
                fillers.append(outproj_windows(wins[2:4]))
            flush_all()

    return nc


def _prep_in_maps(inputs):
    q = np.ascontiguousarray(inputs["query"], dtype=np.float32).reshape(R, D)
    k = np.ascontiguousarray(inputs["key"], dtype=np.float32).reshape(R, D)
    v = np.ascontiguousarray(inputs["value"], dtype=np.float32).reshape(R, D)
    Wq = np.asarray(inputs["Wq"], np.float32)
    Wk = np.asarray(inputs["Wk"], np.float32)
    Wv = np.asarray(inputs["Wv"], np.float32)
    Wo = np.asarray(inputs["Wo"], np.float32)
    bq = np.asarray(inputs["bq"], np.float32)
    bk = np.asarray(inputs["bk"], np.float32)
    bv = np.asarray(inputs["bv"], np.float32)

    xqT = np.ascontiguousarray(q.T).astype(bf16)
    xkT = np.ascontiguousarray(k.T).astype(bf16)
    xvT = np.ascontiguousarray(v.T).astype(bf16)
    WqT = np.ascontiguousarray(Wq.T).astype(bf16)
    WkT = np.ascontiguousarray(Wk.T).astype(bf16)
    WvT = np.ascontiguousarray(Wv.T).astype(bf16)
    WoT = np.ascontiguousarray(Wo.T).astype(bf16)
    tri_m = np.arange(128)[:, None] <= np.arange(128)[None, :]
    tri_h = np.ascontiguousarray(
        np.broadcast_to(tri_m[:, None, :], (128, 2, 128))
    ).astype(bf16)
    id_h = np.eye(128, dtype=np.float32).astype(bf16)

    in_maps = []
    for c in range(NCORES):
        sl = slice(c * 128, (c + 1) * 128)
        in_maps.append(
            {
                "xqT": xqT,
                "xkT": xkT,
                "xvT": xvT,
                "wq": np.ascontiguousarray(WqT[:, sl]),
                "wk": np.ascontiguousarray(WkT[:, sl]),
                "wv": np.ascontiguousarray(WvT[:, sl]),
                "wo": np.ascontiguousarray(WoT[sl, :]),
                "bq": np.ascontiguousarray(bq[sl].reshape(128, 1)),
                "bk": np.ascontiguousarray(bk[sl].reshape(128, 1)),
                "bv": np.ascontiguousarray(bv[sl].reshape(128, 1)),
                "tri": tri_h,
                "ident": id_h,
            }
        )
    return in_maps


def kernel(**inputs) -> np.ndarray:
    nc = _CACHE.get("nc")
    if nc is None:
        nc = _build_program()
        nc.finalize()  # Bacc legalization (register alloc, event-sem splitting)
        _CACHE["nc"] = nc
    in_maps = _prep_in_maps(inputs)
    trace = bool(int(os.environ.get("KERNEL_TRACE", "0")))
    res = run_bass_kernel_spmd(nc, in_maps, list(range(NCORES)), trace=trace)
    _CACHE["last"] = res
    acc = res.results[0]["out"].astype(np.float32)
    for c in range(1, NCORES):
        acc += res.results[c]["out"].astype(np.float32)
    full = acc.T + np.asarray(inputs["bo"], np.float32)[None, :]
    return np.ascontiguousarray(full).reshape(B, S, D)


# revision 37
# speedup vs baseline: 1.0220x; 1.0220x over previous
"""Multi-head attention (B=4, S=2048, D=1024, H=16, causal) on 8 trn2 NeuronCores.

Sharding: tensor-parallel over heads. Core c owns heads {2c, 2c+1} = model dims
[c*128, (c+1)*128).

Per-core pipeline (all matmul inputs bf16, fp32 PSUM accumulation):
  A) Q/K/V projections in transposed layout  X_c [128 dims, rows]
     (lhsT = W.T chunk stationary, rhs = host-pretransposed input stream).
  B) V transposed back to row-major via PE transpose and augmented with a
     64-wide ones block -> V_aug tiles [128 k-rows, 192].
  C) Attention per (batch, 512-q window, 128-k block), causal blocks only:
     scoresT [k,q] via 2-head row-packed matmuls, exp (scale=1/8 folded in,
     no max subtraction - scores are O(1) by construction), triangular mask
     on diagonal blocks, PV accumulation with the ones-augmented V so PSUM
     rows carry both context and the softmax denominator replicated 64x.
     Per-batch clustered normalize: 1/l = exp(-ln l) on ACT (one Ln + one
     Exp over the batch slab -> two act-table loads per batch), one DVE
     mul -> ctxT [dims, q] bf16.
  D) Output projection -> outT [1024 o, q] bf16 partials per core; host
     transposes, upcasts and reduces the 8 partials.

Scheduling: single-pass emission with a filler FIFO. Attention windows run
with one-block lookahead (sc(k+1) issued before pv(k) so the exp on ACT is
off the PE critical path); between attention matmuls the scheduler pops
"filler" PE micro-ops (next batch's projections, previous windows' output
projections) so the PE never idles waiting on ACT/DVE.
"""

import os
import sys
from collections import deque
from contextlib import ExitStack

sys.path.insert(0, "/opt/trn_rl_repo")

import numpy as np
import ml_dtypes

import concourse.bass as bass
import concourse.bacc as bacc
import concourse.mybir as mybir
import concourse.tile as tile
from concourse.bass_utils import run_bass_kernel_spmd

BF = mybir.dt.bfloat16
F32 = mybir.dt.float32
bf16 = ml_dtypes.bfloat16

B, S, D = 4, 2048, 1024
R = B * S  # 8192
NCORES = 8
QW = 512  # q-window
NKB = S // 128  # 16 k-blocks per batch

_CACHE: dict = {}


def _build_program() -> bass.Bass:
    nc = bacc.Bacc(None, num_devices=NCORES)
    xqT = nc.dram_tensor("xqT", [D, R], BF, kind="ExternalInput")
    xkT = nc.dram_tensor("xkT", [D, R], BF, kind="ExternalInput")
    xvT = nc.dram_tensor("xvT", [D, R], BF, kind="ExternalInput")
    wq = nc.dram_tensor("wq", [D, 128], BF, kind="ExternalInput")
    wk = nc.dram_tensor("wk", [D, 128], BF, kind="ExternalInput")
    wv = nc.dram_tensor("wv", [D, 128], BF, kind="ExternalInput")
    wo = nc.dram_tensor("wo", [128, D], BF, kind="ExternalInput")
    bq = nc.dram_tensor("bq", [128, 1], F32, kind="ExternalInput")
    bk = nc.dram_tensor("bk", [128, 1], F32, kind="ExternalInput")
    bv = nc.dram_tensor("bv", [128, 1], F32, kind="ExternalInput")
    tri = nc.dram_tensor("tri", [128, 2, 128], BF, kind="ExternalInput")
    ident = nc.dram_tensor("ident", [128, 128], BF, kind="ExternalInput")
    out_ext = nc.dram_tensor("out", [D, R], BF, kind="ExternalOutput")

    EXP = mybir.ActivationFunctionType.Exp
    LN = mybir.ActivationFunctionType.Ln

    with ExitStack() as ctx:
        tc = ctx.enter_context(tile.TileContext(nc))
        singles = ctx.enter_context(tc.tile_pool(name="singles", bufs=1))
        stage = ctx.enter_context(tc.tile_pool(name="stage", bufs=20))
        qkv = ctx.enter_context(tc.tile_pool(name="qkv", bufs=2))
        vst = ctx.enter_context(tc.tile_pool(name="vst", bufs=3))
        exps = ctx.enter_context(tc.tile_pool(name="exps", bufs=8))
        divp = ctx.enter_context(tc.tile_pool(name="divp", bufs=6))
        outp = ctx.enter_context(tc.tile_pool(name="outp", bufs=2))
        ps_proj = ctx.enter_context(tc.tile_pool(name="ps_proj", bufs=2, space="PSUM"))
        ps_sc = ctx.enter_context(tc.tile_pool(name="ps_sc", bufs=2, space="PSUM"))
        ps_pv = ctx.enter_context(tc.tile_pool(name="ps_pv", bufs=2, space="PSUM"))

        # resident constants
        wq_sb = singles.tile([128, 8, 128], BF, name="wq_sb")
        wk_sb = singles.tile([128, 8, 128], BF, name="wk_sb")
        wv_sb = singles.tile([128, 8, 128], BF, name="wv_sb")
        nc.sync.dma_start(wq_sb[:], wq[:, :].rearrange("(f p) m -> p f m", p=128))
        nc.sync.dma_start(wk_sb[:], wk[:, :].rearrange("(f p) m -> p f m", p=128))
        nc.sync.dma_start(wv_sb[:], wv[:, :].rearrange("(f p) m -> p f m", p=128))
        wo_sb = singles.tile([128, D], BF, name="wo_sb")
        nc.sync.dma_start(wo_sb[:], wo[:, :])
        bq_sb = singles.tile([128, 1], F32, name="bq_sb")
        bk_sb = singles.tile([128, 1], F32, name="bk_sb")
        bv_sb = singles.tile([128, 1], F32, name="bv_sb")
        nc.sync.dma_start(bq_sb[:], bq[:, :])
        nc.sync.dma_start(bk_sb[:], bk[:, :])
        nc.sync.dma_start(bv_sb[:], bv[:, :])
        tri_sb = singles.tile([128, 2, 128], BF, name="tri_sb")
        nc.sync.dma_start(tri_sb[:], tri[:, :, :])
        id_sb = singles.tile([128, 128], BF, name="id_sb")
        nc.sync.dma_start(id_sb[:], ident[:, :])

        warm_sb = singles.tile([128, 512], BF, name="warm_sb")
        nc.vector.memset(warm_sb[:], 0.0)
        warm_ps = ps_proj.tile([128, 512], F32, tag="proj", name="warm_ps")
        for wi in range(24):
            nc.tensor.matmul(
                warm_ps[:],
                warm_sb[:, 0:128],
                warm_sb[:],
                start=(wi == 0),
                stop=(wi == 23),
            )

        tiles = {}
        bslabs = {}
        fillers: deque = deque()

        def pump(n):
            while n > 0 and fillers:
                try:
                    next(fillers[0])
                    n -= 1
                except StopIteration:
                    fillers.popleft()

        def flush_all():
            while fillers:
                try:
                    next(fillers[0])
                except StopIteration:
                    fillers.popleft()

        def alloc_batch(b):
            q_sb = qkv.tile([128, S], BF, tag="q_sb", name=f"q_sb{b}")
            k_sb = qkv.tile([128, S], BF, tag="k_sb", name=f"k_sb{b}")
            v_aug = qkv.tile([128, NKB, 192], BF, tag="v_aug", name=f"v_aug{b}")
            nc.vector.memset(v_aug[:, :, 64:128], 1.0)
            tiles[b] = (q_sb, k_sb, v_aug)

        def projection_microops(b, which, half):
            # which: 0=q, 1=k, 2=v; half: 0/1 = 1024-row sector. Generator:
            # one PE matmul per next(). f-outer over a pair of 512-col psum
            # groups so consecutive matmuls share lhsT.
            if b not in tiles:
                alloc_batch(b)
            q_sb, k_sb, v_aug = tiles[b]
            xT, w_sb, b_sb = (
                (xqT, wq_sb, bq_sb),
                (xkT, wk_sb, bk_sb),
                (xvT, wv_sb, bv_sb),
            )[which]
            st = {}
            for f in range(8):
                s_t = stage.tile([128, S // 2], BF, tag="stage")
                nc.sync.dma_start(
                    s_t[:],
                    xT[
                        f * 128 : (f + 1) * 128,
                        b * S + half * (S // 2) : b * S + (half + 1) * (S // 2),
                    ],
                )
                st[f] = s_t
            pa = ps_proj.tile(
                [128, 512], F32, tag="proj", name=f"pa{b}_{half}_{which}"
            )
            pb = ps_proj.tile(
                [128, 512], F32, tag="proj", name=f"pb{b}_{half}_{which}"
            )
            for f in range(8):
                nc.tensor.matmul(
                    pa[:],
                    w_sb[:, f, :],
                    st[f][:, 0:512],
                    start=(f == 0),
                    stop=(f == 7),
                )
                yield
                nc.tensor.matmul(
                    pb[:],
                    w_sb[:, f, :],
                    st[f][:, 512:1024],
                    start=(f == 0),
                    stop=(f == 7),
                )
                yield
            t0 = 2 * half
            if which == 0:
                nc.vector.tensor_scalar_add(
                    q_sb[:, t0 * 512 : (t0 + 1) * 512], pa[:], bq_sb[:]
                )
                nc.vector.tensor_scalar_add(
                    q_sb[:, (t0 + 1) * 512 : (t0 + 2) * 512], pb[:], bq_sb[:]
                )
            elif which == 1:
                nc.vector.tensor_scalar_add(
                    k_sb[:, t0 * 512 : (t0 + 1) * 512], pa[:], bk_sb[:]
                )
                nc.vector.tensor_scalar_add(
                    k_sb[:, (t0 + 1) * 512 : (t0 + 2) * 512], pb[:], bk_sb[:]
                )
            else:
                v_sts = {}
                for t, ps in ((t0, pa), (t0 + 1, pb)):
                    v_st = vst.tile([128, 512], BF, tag="v_st")
                    nc.vector.tensor_scalar_add(v_st[:], ps[:], bv_sb[:])
                    v_sts[t] = v_st
                for t in (t0, t0 + 1):
                    for s4 in range(4):
                        pst = ps_proj.tile(
                            [128, 128], BF, tag="proj", name=f"pst{b}_{t}_{s4}"
                        )
                        nc.tensor.transpose(
                            pst[:], v_sts[t][:, s4 * 128 : (s4 + 1) * 128], id_sb[:]
                        )
                        tt = t * 4 + s4
                        nc.vector.tensor_copy(v_aug[:, tt, 0:64], pst[:, 0:64])
                        nc.vector.tensor_copy(v_aug[:, tt, 128:192], pst[:, 64:128])
                        yield

        def outproj_windows(wins, act_mod=4):
            # Output projection for one or more q-windows; back-to-back
            # matmuls per ob share the wo lhsT chunk. One DMA per window.
            # act_mod: every act_mod-th PSUM evac copy goes to ACT, rest DVE.
            ots = [
                outp.tile([128, 8, 512], BF, tag="ot", name=f"ot{win}")
                for _, win in wins
            ]
            for ob in range(8):
                pos = []
                for (ctx_t, win), ot in zip(wins, ots):
                    po = ps_proj.tile(
                        [128, 512], F32, tag="proj", name=f"po{win}_{ob}"
                    )
                    nc.tensor.matmul(
                        po[:],
                        wo_sb[:, ob * 128 : (ob + 1) * 128],
                        ctx_t[:],
                        start=True,
                        stop=True,
                    )
                    pos.append((po, ot))
                    yield
                for po, ot in pos:
                    if ob % act_mod == act_mod - 1:
                        nc.scalar.copy(ot[:, ob, :], po[:])
                    else:
                        nc.vector.tensor_copy(ot[:, ob, :], po[:])
            for (ctx_t, win), ot in zip(wins, ots):
                nc.gpsimd.dma_start(
                    out_ext[:, win : win + 512].rearrange("(f p) m -> p f m", p=128),
                    ot[:],
                )

        def emit_attention_qb(b, qb):
            q_sb, k_sb, v_aug = tiles[b]
            nk = 4 * qb + 4  # causal: k-blocks 0 .. 4qb+3
            pv0 = ps_pv.tile([128, 512], F32, tag="pv", name=f"pv0_{b}_{qb}")
            pv1 = ps_pv.tile([128, 512], F32, tag="pv", name=f"pv1_{b}_{qb}")
            ets = {}

            def emit_sc(kblk):
                r = kblk - 4 * qb
                q_lo = max(0, r * 128)
                sc = ps_sc.tile([128, 2, 512], F32, tag="sc")
                for h in range(2):
                    nc.tensor.matmul(
                        sc[:, h, q_lo:512],
                        k_sb[h * 64 : (h + 1) * 64, kblk * 128 : (kblk + 1) * 128],
                        q_sb[
                            h * 64 : (h + 1) * 64,
                            qb * 512 + q_lo : (qb + 1) * 512,
                        ],
                        start=True,
                        stop=True,
                        tile_position=(h * 64, 0),
                    )
                et = exps.tile([128, 2, 512], BF, tag="et")
                nc.scalar.activation(
                    et[:, :, q_lo:512], sc[:, :, q_lo:512], EXP, scale=0.125
                )
                if r >= 0:
                    nc.vector.tensor_mul(
                        et[:, :, q_lo : q_lo + 128],
                        et[:, :, q_lo : q_lo + 128],
                        tri_sb[:],
                    )
                ets[kblk] = (et, q_lo)

            def emit_pv(kblk):
                et, q_lo = ets.pop(kblk)
                for h, pv in ((0, pv0), (1, pv1)):
                    nc.tensor.matmul(
                        pv[:, q_lo:512],
                        v_aug[:, kblk, h * 64 : h * 64 + 128],
                        et[:, h, q_lo:512],
                        start=(kblk == 0),
                        stop=(kblk == nk - 1),
                    )

            emit_sc(0)
            pump(3)
            for kblk in range(nk):
                if kblk + 1 < nk:
                    emit_sc(kblk + 1)
                pump(1)
                emit_pv(kblk)
            # evac: pv0 rows[0:64]=ctx~ h0, rows[64:128]=l h0 (replicated)
            #       pv1 rows[0:64]=l h1,   rows[64:128]=ctx~ h1
            # DVE copies free the pv banks ~2.5us after the last
            # accumulation, landing in per-batch slab tiles (cslb/lslb).
            # The 1/l normalize is ONE Ln + ONE Exp + ONE mul over the whole
            # batch slab: the tile scheduler cannot interleave attention
            # exps between them, so the Ln<->Exp act-table swap happens
            # twice per batch instead of twice per window.
            cslb, lslb = bslabs[b]
            nc.vector.tensor_copy(cslb[0:64, qb, :], pv0[0:64, :])
            nc.vector.tensor_copy(lslb[0:64, qb, :], pv0[64:128, :])
            nc.vector.tensor_copy(cslb[64:128, qb, :], pv1[64:128, :])
            nc.vector.tensor_copy(lslb[64:128, qb, :], pv1[0:64, :])

        def emit_norm_cluster(b, q0, n):
            # One Ln + one Exp + one mul over windows [q0, q0+n) of batch b.
            cslb, lslb = bslabs[b]
            lnw = divp.tile([128, 4, 512], F32, tag="lnw", bufs=3)
            nc.scalar.activation(lnw[:, 0:n, :], lslb[:, q0 : q0 + n, :], LN)
            recb = divp.tile([128, 4, 512], BF, tag="recb", bufs=3)
            nc.scalar.activation(recb[:, 0:n, :], lnw[:, 0:n, :], EXP, scale=-1.0)
            ctxb = divp.tile([128, 4, 512], BF, tag="ctxb", bufs=3)
            nc.vector.tensor_mul(
                ctxb[:, 0:n, :], cslb[:, q0 : q0 + n, :], recb[:, 0:n, :]
            )
            return [(ctxb[:, i, :], b * S + (q0 + i) * 512) for i in range(n)]

        # prologue: batch-0 half-0 projections inline (rows 0:1024 of q/k/v);
        # windows 0-1 only need those, so attention starts while half-1
        # streams in as filler work.
        for which in range(3):
            for _ in projection_microops(0, which, 0):
                pass
        b0h1 = [projection_microops(0, which, 1) for which in range(3)]
        for g in b0h1:
            fillers.append(g)
        for b in range(B):
            if b + 1 < B:
                for which in range(3):
                    for half in range(2):
                        fillers.append(projection_microops(b + 1, which, half))
            bslabs[b] = (
                divp.tile([128, 4, 512], BF, tag="cslb", name=f"cslb{b}", bufs=2),
                divp.tile([128, 4, 512], F32, tag="lslb", name=f"lslb{b}", bufs=2),
            )
            for qb in range(S // QW):
                if b == 0 and qb == 2:
                    # windows 2-3 of batch 0 need half-1 q/k/v: drain the
                    # batch-0 half-1 projection fillers first
                    while any(g in fillers for g in b0h1):
                        pump(1)
                emit_attention_qb(b, qb)
                if b == B - 1 and qb == 1:
                    # last batch has no next-batch projections to fill PE
                    # idle: normalize windows 0-1 early so their output
                    # projection overlaps windows 2-3's attention
                    fillers.append(outproj_windows(emit_norm_cluster(b, 0, 2)))
            if b == B - 1:
                fillers.append(outproj_windows(emit_norm_cluster(b, 2, 2)))
            else:
                wins = emit_norm_cluster(b, 0, 4)
                fillers.append(outproj_windows(wins[0:2]))
                fillers.append(outproj_windows(wins[2:4]))
            flush_all()

    return nc


def _prep_in_maps(inputs):
    q = np.ascontiguousarray(inputs["query"], dtype=np.float32).reshape(R, D)
    k = np.ascontiguousarray(inputs["key"], dtype=np.float32).reshape(R, D)
    v = np.ascontiguousarray(inputs["value"], dtype=np.float32).reshape(R, D)
    Wq = np.asarray(inputs["Wq"], np.float32)
    Wk = np.asarray(inputs["Wk"], np.float32)
    Wv = np.asarray(inputs["Wv"], np.float32)
    Wo = np.asarray(inputs["Wo"], np.float32)
    bq = np.asarray(inputs["bq"], np.float32)
    bk = np.asarray(inputs["bk"], np.float32)
    bv = np.asarray(inputs["bv"], np.float32)

    xqT = np.ascontiguousarray(q.T).astype(bf16)
    xkT = np.ascontiguousarray(k.T).astype(bf16)
    xvT = np.ascontiguousarray(v.T).astype(bf16)
    WqT = np.ascontiguousarray(Wq.T).astype(bf16)
    WkT = np.ascontiguousarray(Wk.T).astype(bf16)
    WvT = np.ascontiguousarray(Wv.T).astype(bf16)
    WoT = np.ascontiguousarray(Wo.T).astype(bf16)
    tri_m = np.arange(128)[:, None] <= np.arange(128)[None, :]
    tri_h = np.ascontiguousarray(
        np.broadcast_to(tri_m[:, None, :], (128, 2, 128))
    ).astype(bf16)
    id_h = np.eye(128, dtype=np.float32).astype(bf16)

    in_maps = []
    for c in range(NCORES):
        sl = slice(c * 128, (c + 1) * 128)
        in_maps.append(
            {
                "xqT": xqT,
                "xkT": xkT,
                "xvT": xvT,
                "wq": np.ascontiguousarray(WqT[:, sl]),
                "wk": np.ascontiguousarray(WkT[:, sl]),
                "wv": np.ascontiguousarray(WvT[:, sl]),
                "wo": np.ascontiguousarray(WoT[sl, :]),
                "bq": np.ascontiguousarray(bq[sl].reshape(128, 1)),
                "bk": np.ascontiguousarray(bk[sl].reshape(128, 1)),
                "bv": np.ascontiguousarray(bv[sl].reshape(128, 1)),
                "tri": tri_h,
                "ident": id_h,
            }
        )
    return in_maps


def kernel(**inputs) -> np.ndarray:
    nc = _CACHE.get("nc")
    if nc is None:
        nc = _build_program()
        nc.finalize()  # Bacc legalization (register alloc, event-sem splitting)
        _CACHE["nc"] = nc
    in_maps = _prep_in_maps(inputs)
    trace = bool(int(os.environ.get("KERNEL_TRACE", "0")))
    res = run_bass_kernel_spmd(nc, in_maps, list(range(NCORES)), trace=trace)
    _CACHE["last"] = res
    acc = res.results[0]["out"].astype(np.float32)
    for c in range(1, NCORES):
        acc += res.results[c]["out"].astype(np.float32)
    full = acc.T + np.asarray(inputs["bo"], np.float32)[None, :]
    return np.ascontiguousarray(full).reshape(B, S, D)


# revision 40
# speedup vs baseline: 1.0714x; 1.0484x over previous
"""Multi-head attention (B=4, S=2048, D=1024, H=16, causal) on 8 trn2 NeuronCores.

Sharding: tensor-parallel over heads. Core c owns heads {2c, 2c+1} = model dims
[c*128, (c+1)*128).

Per-core pipeline (all matmul inputs bf16, fp32 PSUM accumulation):
  A) Q/K/V projections in transposed layout  X_c [128 dims, rows]
     (lhsT = W.T chunk stationary, rhs = host-pretransposed input stream).
  B) V transposed back to row-major via PE transpose and augmented with a
     64-wide ones block -> V_aug tiles [128 k-rows, 192].
  C) Attention per (batch, 512-q window, 128-k block), causal blocks only:
     scoresT [k,q] via 2-head row-packed matmuls, exp (scale=1/8 folded in,
     no max subtraction - scores are O(1) by construction), triangular mask
     on diagonal blocks, PV accumulation with the ones-augmented V so PSUM
     rows carry both context and the softmax denominator replicated 64x.
     Per-batch clustered normalize: 1/l = exp(-ln l) on ACT (one Ln + one
     Exp over the batch slab -> two act-table loads per batch), one DVE
     mul -> ctxT [dims, q] bf16.
  D) Output projection -> outT [1024 o, q] bf16 partials per core; host
     transposes, upcasts and reduces the 8 partials.

Scheduling: single-pass emission with a filler FIFO. Attention windows run
with one-block lookahead (sc(k+1) issued before pv(k) so the exp on ACT is
off the PE critical path); between attention matmuls the scheduler pops
"filler" PE micro-ops (next batch's projections, previous windows' output
projections) so the PE never idles waiting on ACT/DVE.
"""

import os
import sys
from collections import deque
from contextlib import ExitStack

sys.path.insert(0, "/opt/trn_rl_repo")

import numpy as np
import ml_dtypes

import concourse.bass as bass
import concourse.bacc as bacc
import concourse.mybir as mybir
import concourse.tile as tile
from concourse.bass_utils import run_bass_kernel_spmd

BF = mybir.dt.bfloat16
F32 = mybir.dt.float32
bf16 = ml_dtypes.bfloat16

B, S, D = 4, 2048, 1024
R = B * S  # 8192
NCORES = 8
QW = 512  # q-window
NKB = S // 128  # 16 k-blocks per batch

_CACHE: dict = {}


def _build_program() -> bass.Bass:
    nc = bacc.Bacc(None, num_devices=NCORES)
    xqT = nc.dram_tensor("xqT", [D, R], BF, kind="ExternalInput")
    xkT = nc.dram_tensor("xkT", [D, R], BF, kind="ExternalInput")
    xvT = nc.dram_tensor("xvT", [D, R], BF, kind="ExternalInput")
    wq = nc.dram_tensor("wq", [D, 128], BF, kind="ExternalInput")
    wk = nc.dram_tensor("wk", [D, 128], BF, kind="ExternalInput")
    wv = nc.dram_tensor("wv", [D, 128], BF, kind="ExternalInput")
    wo = nc.dram_tensor("wo", [128, D], BF, kind="ExternalInput")
    bq = nc.dram_tensor("bq", [128, 1], F32, kind="ExternalInput")
    bk = nc.dram_tensor("bk", [128, 1], F32, kind="ExternalInput")
    bv = nc.dram_tensor("bv", [128, 1], F32, kind="ExternalInput")
    tri = nc.dram_tensor("tri", [128, 2, 128], BF, kind="ExternalInput")
    ident = nc.dram_tensor("ident", [128, 128], BF, kind="ExternalInput")
    out_ext = nc.dram_tensor("out", [D, R], BF, kind="ExternalOutput")

    EXP = mybir.ActivationFunctionType.Exp
    LN = mybir.ActivationFunctionType.Ln

    with ExitStack() as ctx:
        tc = ctx.enter_context(tile.TileContext(nc))
        singles = ctx.enter_context(tc.tile_pool(name="singles", bufs=1))
        stage = ctx.enter_context(tc.tile_pool(name="stage", bufs=20))
        qkv = ctx.enter_context(tc.tile_pool(name="qkv", bufs=2))
        vst = ctx.enter_context(tc.tile_pool(name="vst", bufs=3))
        exps = ctx.enter_context(tc.tile_pool(name="exps", bufs=8))
        divp = ctx.enter_context(tc.tile_pool(name="divp", bufs=6))
        outp = ctx.enter_context(tc.tile_pool(name="outp", bufs=3))
        ps_proj = ctx.enter_context(tc.tile_pool(name="ps_proj", bufs=2, space="PSUM"))
        ps_sc = ctx.enter_context(tc.tile_pool(name="ps_sc", bufs=2, space="PSUM"))
        ps_pv = ctx.enter_context(tc.tile_pool(name="ps_pv", bufs=2, space="PSUM"))

        # resident constants
        wq_sb = singles.tile([128, 8, 128], BF, name="wq_sb")
        wk_sb = singles.tile([128, 8, 128], BF, name="wk_sb")
        wv_sb = singles.tile([128, 8, 128], BF, name="wv_sb")
        nc.sync.dma_start(wq_sb[:], wq[:, :].rearrange("(f p) m -> p f m", p=128))
        nc.sync.dma_start(wk_sb[:], wk[:, :].rearrange("(f p) m -> p f m", p=128))
        nc.sync.dma_start(wv_sb[:], wv[:, :].rearrange("(f p) m -> p f m", p=128))
        wo_sb = singles.tile([128, D], BF, name="wo_sb")
        nc.sync.dma_start(wo_sb[:], wo[:, :])
        bq_sb = singles.tile([128, 1], F32, name="bq_sb")
        bk_sb = singles.tile([128, 1], F32, name="bk_sb")
        bv_sb = singles.tile([128, 1], F32, name="bv_sb")
        nc.sync.dma_start(bq_sb[:], bq[:, :])
        nc.sync.dma_start(bk_sb[:], bk[:, :])
        nc.sync.dma_start(bv_sb[:], bv[:, :])
        tri_sb = singles.tile([128, 2, 128], BF, name="tri_sb")
        nc.sync.dma_start(tri_sb[:], tri[:, :, :])
        id_sb = singles.tile([128, 128], BF, name="id_sb")
        nc.sync.dma_start(id_sb[:], ident[:, :])

        warm_sb = singles.tile([128, 512], BF, name="warm_sb")
        nc.vector.memset(warm_sb[:], 0.0)
        warm_ps = ps_proj.tile([128, 512], F32, tag="proj", name="warm_ps")
        for wi in range(24):
            nc.tensor.matmul(
                warm_ps[:],
                warm_sb[:, 0:128],
                warm_sb[:],
                start=(wi == 0),
                stop=(wi == 23),
            )

        tiles = {}
        bslabs = {}
        fillers: deque = deque()

        def pump(n):
            while n > 0 and fillers:
                try:
                    next(fillers[0])
                    n -= 1
                except StopIteration:
                    fillers.popleft()

        def flush_all():
            while fillers:
                try:
                    next(fillers[0])
                except StopIteration:
                    fillers.popleft()

        def alloc_batch(b):
            q_sb = qkv.tile([128, S], BF, tag="q_sb", name=f"q_sb{b}")
            k_sb = qkv.tile([128, S], BF, tag="k_sb", name=f"k_sb{b}")
            v_aug = qkv.tile([128, NKB, 192], BF, tag="v_aug", name=f"v_aug{b}")
            nc.vector.memset(v_aug[:, :, 64:128], 1.0)
            tiles[b] = (q_sb, k_sb, v_aug)

        def projection_microops(b, which, half):
            # which: 0=q, 1=k, 2=v; half: 0/1 = 1024-row sector. Generator:
            # one PE matmul per next(). f-outer over a pair of 512-col psum
            # groups so consecutive matmuls share lhsT.
            if b not in tiles:
                alloc_batch(b)
            q_sb, k_sb, v_aug = tiles[b]
            xT, w_sb, b_sb = (
                (xqT, wq_sb, bq_sb),
                (xkT, wk_sb, bk_sb),
                (xvT, wv_sb, bv_sb),
            )[which]
            st = {}
            for f in range(8):
                s_t = stage.tile([128, S // 2], BF, tag="stage")
                nc.sync.dma_start(
                    s_t[:],
                    xT[
                        f * 128 : (f + 1) * 128,
                        b * S + half * (S // 2) : b * S + (half + 1) * (S // 2),
                    ],
                )
                st[f] = s_t
            pa = ps_proj.tile(
                [128, 512], F32, tag="proj", name=f"pa{b}_{half}_{which}"
            )
            pb = ps_proj.tile(
                [128, 512], F32, tag="proj", name=f"pb{b}_{half}_{which}"
            )
            for f in range(8):
                nc.tensor.matmul(
                    pa[:],
                    w_sb[:, f, :],
                    st[f][:, 0:512],
                    start=(f == 0),
                    stop=(f == 7),
                )
                yield
                nc.tensor.matmul(
                    pb[:],
                    w_sb[:, f, :],
                    st[f][:, 512:1024],
                    start=(f == 0),
                    stop=(f == 7),
                )
                yield
            t0 = 2 * half
            if which == 0:
                nc.vector.tensor_scalar_add(
                    q_sb[:, t0 * 512 : (t0 + 1) * 512], pa[:], bq_sb[:]
                )
                nc.vector.tensor_scalar_add(
                    q_sb[:, (t0 + 1) * 512 : (t0 + 2) * 512], pb[:], bq_sb[:]
                )
            elif which == 1:
                nc.vector.tensor_scalar_add(
                    k_sb[:, t0 * 512 : (t0 + 1) * 512], pa[:], bk_sb[:]
                )
                nc.vector.tensor_scalar_add(
                    k_sb[:, (t0 + 1) * 512 : (t0 + 2) * 512], pb[:], bk_sb[:]
                )
            else:
                v_sts = {}
                for t, ps in ((t0, pa), (t0 + 1, pb)):
                    v_st = vst.tile([128, 512], BF, tag="v_st")
                    nc.vector.tensor_scalar_add(v_st[:], ps[:], bv_sb[:])
                    v_sts[t] = v_st
                for t in (t0, t0 + 1):
                    for s4 in range(4):
                        pst = ps_proj.tile(
                            [128, 128], BF, tag="proj", name=f"pst{b}_{t}_{s4}"
                        )
                        nc.tensor.transpose(
                            pst[:], v_sts[t][:, s4 * 128 : (s4 + 1) * 128], id_sb[:]
                        )
                        tt = t * 4 + s4
                        nc.vector.tensor_copy(v_aug[:, tt, 0:64], pst[:, 0:64])
                        nc.vector.tensor_copy(v_aug[:, tt, 128:192], pst[:, 64:128])
                        yield

        def outproj_windows(wins, act_mod=4):
            # Output projection for one or more q-windows; back-to-back
            # matmuls per ob share the wo lhsT chunk. One DMA per window.
            # act_mod: every act_mod-th PSUM evac copy goes to ACT, rest DVE.
            ots = [
                outp.tile([128, 8, 512], BF, tag="ot", name=f"ot{win}")
                for _, win in wins
            ]
            for ob in range(8):
                pos = []
                for (ctx_t, win), ot in zip(wins, ots):
                    po = ps_proj.tile(
                        [128, 512], F32, tag="proj", name=f"po{win}_{ob}"
                    )
                    nc.tensor.matmul(
                        po[:],
                        wo_sb[:, ob * 128 : (ob + 1) * 128],
                        ctx_t[:],
                        start=True,
                        stop=True,
                    )
                    pos.append((po, ot))
                    yield
                for po, ot in pos:
                    if ob % act_mod == act_mod - 1:
                        nc.scalar.copy(ot[:, ob, :], po[:])
                    else:
                        nc.vector.tensor_copy(ot[:, ob, :], po[:])
            for (ctx_t, win), ot in zip(wins, ots):
                nc.gpsimd.dma_start(
                    out_ext[:, win : win + 512].rearrange("(f p) m -> p f m", p=128),
                    ot[:],
                )

        def emit_attention_qb(b, qb):
            q_sb, k_sb, v_aug = tiles[b]
            nk = 4 * qb + 4  # causal: k-blocks 0 .. 4qb+3
            pv0 = ps_pv.tile([128, 512], F32, tag="pv", name=f"pv0_{b}_{qb}")
            pv1 = ps_pv.tile([128, 512], F32, tag="pv", name=f"pv1_{b}_{qb}")
            ets = {}

            def emit_sc(kblk):
                r = kblk - 4 * qb
                q_lo = max(0, r * 128)
                sc = ps_sc.tile([128, 2, 512], F32, tag="sc")
                for h in range(2):
                    nc.tensor.matmul(
                        sc[:, h, q_lo:512],
                        k_sb[h * 64 : (h + 1) * 64, kblk * 128 : (kblk + 1) * 128],
                        q_sb[
                            h * 64 : (h + 1) * 64,
                            qb * 512 + q_lo : (qb + 1) * 512,
                        ],
                        start=True,
                        stop=True,
                        tile_position=(h * 64, 0),
                    )
                et = exps.tile([128, 2, 512], BF, tag="et")
                nc.scalar.activation(
                    et[:, :, q_lo:512], sc[:, :, q_lo:512], EXP, scale=0.125
                )
                if r >= 0:
                    nc.vector.tensor_mul(
                        et[:, :, q_lo : q_lo + 128],
                        et[:, :, q_lo : q_lo + 128],
                        tri_sb[:],
                    )
                ets[kblk] = (et, q_lo)

            def emit_pv(kblk):
                et, q_lo = ets.pop(kblk)
                for h, pv in ((0, pv0), (1, pv1)):
                    nc.tensor.matmul(
                        pv[:, q_lo:512],
                        v_aug[:, kblk, h * 64 : h * 64 + 128],
                        et[:, h, q_lo:512],
                        start=(kblk == 0),
                        stop=(kblk == nk - 1),
                    )

            emit_sc(0)
            pump(3)
            for kblk in range(nk):
                if kblk + 1 < nk:
                    emit_sc(kblk + 1)
                pump(1)
                emit_pv(kblk)
            # evac: pv0 rows[0:64]=ctx~ h0, rows[64:128]=l h0 (replicated)
            #       pv1 rows[0:64]=l h1,   rows[64:128]=ctx~ h1
            # DVE copies free the pv banks ~2.5us after the last
            # accumulation, landing in per-batch slab tiles (cslb/lslb).
            # The 1/l normalize is ONE Ln + ONE Exp + ONE mul over the whole
            # batch slab: the tile scheduler cannot interleave attention
            # exps between them, so the Ln<->Exp act-table swap happens
            # twice per batch instead of twice per window.
            cslb, lslb = bslabs[b]
            nc.vector.tensor_copy(cslb[0:64, qb, :], pv0[0:64, :])
            nc.vector.tensor_copy(lslb[0:64, qb, :], pv0[64:128, :])
            nc.vector.tensor_copy(cslb[64:128, qb, :], pv1[64:128, :])
            nc.vector.tensor_copy(lslb[64:128, qb, :], pv1[0:64, :])

        def emit_norm_cluster(b, q0, n):
            # One Ln + one Exp + one mul over windows [q0, q0+n) of batch b.
            cslb, lslb = bslabs[b]
            lnw = divp.tile([128, 4, 512], F32, tag="lnw", bufs=3)
            nc.scalar.activation(lnw[:, 0:n, :], lslb[:, q0 : q0 + n, :], LN)
            recb = divp.tile([128, 4, 512], BF, tag="recb", bufs=3)
            nc.scalar.activation(recb[:, 0:n, :], lnw[:, 0:n, :], EXP, scale=-1.0)
            ctxb = divp.tile([128, 4, 512], BF, tag="ctxb", bufs=3)
            nc.vector.tensor_mul(
                ctxb[:, 0:n, :], cslb[:, q0 : q0 + n, :], recb[:, 0:n, :]
            )
            return [(ctxb[:, i, :], b * S + (q0 + i) * 512) for i in range(n)]

        # prologue: batch-0 half-0 projections inline (rows 0:1024 of q/k/v);
        # windows 0-1 only need those, so attention starts while half-1
        # streams in as filler work.
        for which in range(3):
            for _ in projection_microops(0, which, 0):
                pass
        b0h1 = [projection_microops(0, which, 1) for which in range(3)]
        for g in b0h1:
            fillers.append(g)
        deferred = None
        for b in range(B):
            if b + 1 < B:
                for which in range(3):
                    for half in range(2):
                        fillers.append(projection_microops(b + 1, which, half))
            bslabs[b] = (
                divp.tile([128, 4, 512], BF, tag="cslb", name=f"cslb{b}", bufs=2),
                divp.tile([128, 4, 512], F32, tag="lslb", name=f"lslb{b}", bufs=2),
            )
            for qb in range(S // QW):
                if b == 0 and qb == 2:
                    # windows 2-3 of batch 0 need half-1 q/k/v: drain the
                    # batch-0 half-1 projection fillers first
                    while any(g in fillers for g in b0h1):
                        pump(1)
                emit_attention_qb(b, qb)
                if b == B - 1 and qb == 1:
                    # last batch has no next-batch projections to fill PE
                    # idle: normalize windows 0-1 early so their output
                    # projection overlaps windows 2-3's attention
                    fillers.append(outproj_windows(emit_norm_cluster(b, 0, 2)))
                if b == B - 1 and qb == 2 and deferred is not None:
                    # batch-2's held-back output projection: tail filler
                    fillers.append(deferred)
                    deferred = None
            if b == B - 1:
                fillers.append(
                    outproj_windows(emit_norm_cluster(b, 2, 2), act_mod=2)
                )
            else:
                wins = emit_norm_cluster(b, 0, 4)
                fillers.append(outproj_windows(wins[0:2]))
                g = outproj_windows(wins[2:4])
                if b == B - 2:
                    deferred = g
                else:
                    fillers.append(g)
            flush_all()

    return nc


def _prep_in_maps(inputs):
    q = np.ascontiguousarray(inputs["query"], dtype=np.float32).reshape(R, D)
    k = np.ascontiguousarray(inputs["key"], dtype=np.float32).reshape(R, D)
    v = np.ascontiguousarray(inputs["value"], dtype=np.float32).reshape(R, D)
    Wq = np.asarray(inputs["Wq"], np.float32)
    Wk = np.asarray(inputs["Wk"], np.float32)
    Wv = np.asarray(inputs["Wv"], np.float32)
    Wo = np.asarray(inputs["Wo"], np.float32)
    bq = np.asarray(inputs["bq"], np.float32)
    bk = np.asarray(inputs["bk"], np.float32)
    bv = np.asarray(inputs["bv"], np.float32)

    xqT = np.ascontiguousarray(q.T).astype(bf16)
    xkT = np.ascontiguousarray(k.T).astype(bf16)
    xvT = np.ascontiguousarray(v.T).astype(bf16)
    WqT = np.ascontiguousarray(Wq.T).astype(bf16)
    WkT = np.ascontiguousarray(Wk.T).astype(bf16)
    WvT = np.ascontiguousarray(Wv.T).astype(bf16)
    WoT = np.ascontiguousarray(Wo.T).astype(bf16)
    tri_m = np.arange(128)[:, None] <= np.arange(128)[None, :]
    tri_h = np.ascontiguousarray(
        np.broadcast_to(tri_m[:, None, :], (128, 2, 128))
    ).astype(bf16)
    id_h = np.eye(128, dtype=np.float32).astype(bf16)

    in_maps = []
    for c in range(NCORES):
        sl = slice(c * 128, (c + 1) * 128)
        in_maps.append(
            {
                "xqT": xqT,
                "xkT": xkT,
                "xvT": xvT,
                "wq": np.ascontiguousarray(WqT[:, sl]),
                "wk": np.ascontiguousarray(WkT[:, sl]),
                "wv": np.ascontiguousarray(WvT[:, sl]),
                "wo": np.ascontiguousarray(WoT[sl, :]),
                "bq": np.ascontiguousarray(bq[sl].reshape(128, 1)),
                "bk": np.ascontiguousarray(bk[sl].reshape(128, 1)),
                "bv": np.ascontiguousarray(bv[sl].reshape(128, 1)),
                "tri": tri_h,
                "ident": id_h,
            }
        )
    return in_maps


def kernel(**inputs) -> np.ndarray:
    nc = _CACHE.get("nc")
    if nc is None:
        nc = _build_program()
        nc.finalize()  # Bacc legalization (register alloc, event-sem splitting)
        _CACHE["nc"] = nc
    in_maps = _prep_in_maps(inputs)
    trace = bool(int(os.environ.get("KERNEL_TRACE", "0")))
    res = run_bass_kernel_spmd(nc, in_maps, list(range(NCORES)), trace=trace)
    _CACHE["last"] = res
    acc = res.results[0]["out"].astype(np.float32)
    for c in range(1, NCORES):
        acc += res.results[c]["out"].astype(np.float32)
    full = acc.T + np.asarray(inputs["bo"], np.float32)[None, :]
    return np.ascontiguousarray(full).reshape(B, S, D)
